# revision 13
# baseline (speedup 1.0000x reference)
"""Trainium2 Bass kernel for batched tanh-attention flat-softmax.

Computes, per batch b:
    Q = query[b] @ W_query            # [S, DK]
    K = query[b] @ W_key              # [S, DK]
    s = tanh(Q @ K.T) * 10            # [S, S]
    s[diag] = -inf                    # (additive -1e8 in the reference)
    out[b]  = softmax(s.flatten())    # [S*S]

Sharding: data-parallel over batch across 8 NeuronCores (6 batches per
core); W_query/W_key replicated. No cross-core communication.

Since tanh(x)*10 is bounded in [-10, 10], softmax needs no max
subtraction: out = exp(10*tanh(s)) / sum(exp(10*tanh(s))), and the
diagonal is forced to exp(-1e5) == 0 by clamping the tanh output to
-1e4 on the diagonal before the exp.
"""

import numpy as np

import concourse.bass as bass
import concourse.bass_isa as bass_isa
import concourse.mybir as mybir
import concourse.tile as tile
from concourse import bacc
from concourse.bass_utils import run_bass_kernel_spmd
from concourse.masks import make_identity

# Problem shape (hardcoded; kernel.py must be self-contained).
B = 48
S = 1024
D = 128
DK = 64
N_CORES = 8
BPC = B // N_CORES  # batches per core
P = 128             # SBUF partitions
NQ = S // P         # q-row chunks per batch
F32 = mybir.dt.float32
F16 = mybir.dt.float16

TANH_CLIP = 10.0
DIAG_NEG = -1.0e4   # exp(10 * -1e4) underflows to exactly 0 in fp32


def build_bass() -> bass.Bass:
    nc = bacc.Bacc(None, target_bir_lowering=False)

    q_d = nc.dram_tensor("query", [BPC, S, D], F32, kind="ExternalInput")
    wq_d = nc.dram_tensor("W_query", [D, DK], F32, kind="ExternalInput")
    wk_d = nc.dram_tensor("W_key", [D, DK], F32, kind="ExternalInput")
    out_d = nc.dram_tensor("out", [BPC, S, S], F32, kind="ExternalOutput")

    with tile.TileContext(nc) as tc:
        with (
            tc.tile_pool(name="singles", bufs=1) as singles,
            tc.tile_pool(name="qload", bufs=2) as qload,
            tc.tile_pool(name="qtp", bufs=2) as qtp,
            tc.tile_pool(name="projsb", bufs=2) as projsb,
            tc.tile_pool(name="tbuf", bufs=2) as tbuf,
            tc.tile_pool(name="small", bufs=2) as small,
            tc.tile_pool(name="ps_tp", bufs=2, space="PSUM") as ps_tp,
            tc.tile_pool(name="ps_proj", bufs=1, space="PSUM") as ps_proj,
            tc.tile_pool(name="ps_sc", bufs=2, space="PSUM") as ps_sc,
        ):
            # --- one-time setup ---
            ident = singles.tile([P, P], F32)
            make_identity(nc, ident)

            # Diagonal clamp mask: min(t, mask) leaves off-diagonal t
            # untouched (mask=+3e38) and forces the diagonal to -1e4.
            dmask = singles.tile([P, P], F32)
            nc.vector.memset(dmask, 3.0e38)
            nc.gpsimd.affine_select(
                out=dmask,
                in_=dmask,
                compare_op=mybir.AluOpType.not_equal,
                fill=DIAG_NEG,
                base=0,
                pattern=[[-1, P]],
                channel_multiplier=1,
            )

            wq_sb = singles.tile([D, DK], F32)
            nc.sync.dma_start(wq_sb, wq_d[:, :])
            wk_sb = singles.tile([D, DK], F32)
            nc.sync.dma_start(wk_sb, wk_d[:, :])

            for b in range(BPC):
                # --- load query[b] as [p, n, d], s = n*128 + p ---
                q_sb = qload.tile([P, NQ, D], F32)
                nc.sync.dma_start(
                    q_sb, q_d[b].rearrange("(n p) d -> p n d", p=P)
                )

                # --- transpose to queryT [d, (n p)] = [128, 1024] ---
                qT = qtp.tile([D, NQ, P], F32)
                for n in range(NQ):
                    tp_ps = ps_tp.tile([P, P], F32)
                    nc.tensor.transpose(tp_ps, q_sb[:, n], ident)
                    nc.vector.tensor_copy(qT[:, n], tp_ps)

                # --- projections QT/KT = W.T @ queryT, fp32 psum [64, 1024].
                # Split each into fp16 hi+lo for fast, near-fp32 scores:
                #   scores = Qh.T Kh + Ql.T Kh + Qh.T Kl   (Ql.T Kl ~2^-22, dropped)
                # qstack = [Qh; Ql] on 128 partitions; khh = [Kh; Kh]; kl = Kl.
                qproj_ps = ps_proj.tile([DK, S], F32, tag="proj")
                nc.tensor.matmul(qproj_ps[:, 0:512], wq_sb, qT[:, 0:4])
                nc.tensor.matmul(qproj_ps[:, 512:1024], wq_sb, qT[:, 4:8])
                qstack = projsb.tile([P, S], F16, tag="qstack")
                nc.vector.tensor_copy(qstack[0:DK], qproj_ps)
                nc.vector.tensor_tensor(
                    qstack[DK:P], qproj_ps, qstack[0:DK], mybir.AluOpType.subtract
                )

                kproj_ps = ps_proj.tile([DK, S], F32, tag="proj")
                nc.tensor.matmul(kproj_ps[:, 0:512], wk_sb, qT[:, 0:4])
                nc.tensor.matmul(kproj_ps[:, 512:1024], wk_sb, qT[:, 4:8])
                khh = projsb.tile([P, S], F16, tag="khh")
                nc.vector.tensor_copy(khh[0:DK], kproj_ps)
                nc.vector.tensor_copy(khh[DK:P], kproj_ps)
                kl = projsb.tile([DK, S], F16, tag="kl")
                nc.vector.tensor_tensor(
                    kl, kproj_ps, khh[0:DK], mybir.AluOpType.subtract
                )

                # --- scores + tanh per 128-row chunk ---
                t_sb = tbuf.tile([P, NQ, S], F32, tag="t")
                for qc in range(NQ):
                    sc_ps = ps_sc.tile([P, S], F32, tag="sc")
                    lhsT = qstack[:, qc * P:(qc + 1) * P]
                    lhsT_h = qstack[0:DK, qc * P:(qc + 1) * P]
                    for h in range(2):
                        cols = slice(h * 512, (h + 1) * 512)
                        nc.tensor.matmul(
                            sc_ps[:, cols], lhsT, khh[:, cols],
                            start=True, stop=False,
                        )
                        nc.tensor.matmul(
                            sc_ps[:, cols], lhsT_h, kl[:, cols],
                            start=False, stop=True,
                        )
                    nc.scalar.activation(
                        out=t_sb[:, qc],
                        in_=sc_ps,
                        func=mybir.ActivationFunctionType.Tanh,
                    )
                    # clamp this chunk's diagonal block to -1e4
                    blk = t_sb[:, qc, qc * P:(qc + 1) * P]
                    nc.vector.tensor_tensor(blk, blk, dmask, mybir.AluOpType.min)

                # --- exp(10*t) in place, with per-partition row sums ---
                rs = small.tile([P, 1], F32, tag="rs")
                nc.scalar.activation(
                    out=t_sb,
                    in_=t_sb,
                    func=mybir.ActivationFunctionType.Exp,
                    scale=TANH_CLIP,
                    accum_out=rs,
                )

                # --- Z = sum over partitions; rz = 1/Z broadcast [128,1] ---
                zall = small.tile([P, 1], F32, tag="zall")
                nc.gpsimd.partition_all_reduce(
                    zall, rs, channels=P, reduce_op=bass_isa.ReduceOp.add
                )
                rz = small.tile([P, 1], F32, tag="rz")
                nc.vector.reciprocal(rz, zall)

                # --- normalize in place (gpsimd; DVE is busier) and store ---
                nc.gpsimd.tensor_scalar_mul(t_sb, t_sb, rz)
                nc.sync.dma_start(
                    out_d[b].rearrange("(n p) s -> p n s", p=P), t_sb
                )

    nc.compile()
    return nc


_CACHED_NC = None


def kernel(**inputs: np.ndarray) -> np.ndarray:
    global _CACHED_NC
    query = np.ascontiguousarray(np.asarray(inputs["query"], dtype=np.float32))
    wq = np.ascontiguousarray(np.asarray(inputs["W_query"], dtype=np.float32))
    wk = np.ascontiguousarray(np.asarray(inputs["W_key"], dtype=np.float32))
    assert query.shape == (B, S, D), query.shape

    if _CACHED_NC is None:
        _CACHED_NC = build_bass()
    nc = _CACHED_NC

    in_maps = [
        {
            "query": query[c * BPC:(c + 1) * BPC],
            "W_query": wq,
            "W_key": wk,
        }
        for c in range(N_CORES)
    ]
    res = run_bass_kernel_spmd(nc, in_maps, core_ids=list(range(N_CORES)))
    out = np.concatenate(
        [r["out"].reshape(BPC, S * S) for r in res.results], axis=0
    )
    return out


# revision 16
# speedup vs baseline: 4.6864x; 4.6864x over previous
"""Trainium2 Bass kernel for batched tanh-attention flat-softmax.

Computes, per batch b:
    Q = query[b] @ W_query            # [S, DK]
    K = query[b] @ W_key              # [S, DK]
    s = tanh(Q @ K.T) * 10            # [S, S]
    s[diag] = -inf                    # (additive -1e8 in the reference)
    out[b]  = softmax(s.flatten())    # [S*S]

Sharding: data-parallel over batch across 8 NeuronCores (6 batches per
core); W_query/W_key replicated. No cross-core communication.

Since tanh(x)*10 is bounded in [-10, 10], softmax needs no max
subtraction: out = exp(10*tanh(s)) / sum(exp(10*tanh(s))), and the
diagonal is forced to exp(-1e5) == 0 by clamping the tanh output to
-1e4 on the diagonal before the exp.
"""

import numpy as np

import concourse.bass as bass
import concourse.bass_isa as bass_isa
import concourse.mybir as mybir
import concourse.tile as tile
from concourse import bacc
from concourse.bass_utils import run_bass_kernel_spmd
from concourse.masks import make_identity

# Problem shape (hardcoded; kernel.py must be self-contained).
B = 48
S = 1024
D = 128
DK = 64
N_CORES = 8
BPC = B // N_CORES  # batches per core
P = 128             # SBUF partitions
NQ = S // P         # q-row chunks per batch
F32 = mybir.dt.float32
BF16 = mybir.dt.bfloat16

TANH_CLIP = 10.0
DIAG_NEG = -1.0e4   # exp(10 * -1e4) underflows to exactly 0 in fp32


def build_bass() -> bass.Bass:
    nc = bacc.Bacc(None, target_bir_lowering=False)

    q_d = nc.dram_tensor("query", [BPC, S, D], F32, kind="ExternalInput")
    wq_d = nc.dram_tensor("W_query", [D, DK], F32, kind="ExternalInput")
    wk_d = nc.dram_tensor("W_key", [D, DK], F32, kind="ExternalInput")
    out_d = nc.dram_tensor("out", [BPC, S, S], F32, kind="ExternalOutput")

    with tile.TileContext(nc) as tc:
        with (
            tc.tile_pool(name="singles", bufs=1) as singles,
            tc.tile_pool(name="qload", bufs=2) as qload,
            tc.tile_pool(name="qtp", bufs=2) as qtp,
            tc.tile_pool(name="projsb", bufs=2) as projsb,
            tc.tile_pool(name="tbuf", bufs=2) as tbuf,
            tc.tile_pool(name="small", bufs=2) as small,
            tc.tile_pool(name="ps_tp", bufs=2, space="PSUM") as ps_tp,
            tc.tile_pool(name="ps_proj", bufs=1, space="PSUM") as ps_proj,
            tc.tile_pool(name="ps_sc", bufs=2, space="PSUM") as ps_sc,
        ):
            # --- one-time setup ---
            ident = singles.tile([P, P], F32)
            make_identity(nc, ident)

            # Diagonal clamp mask: min(t, mask) leaves off-diagonal t
            # untouched (mask=+3e38) and forces the diagonal to -1e4.
            dmask = singles.tile([P, P], F32)
            nc.vector.memset(dmask, 3.0e38)
            nc.gpsimd.affine_select(
                out=dmask,
                in_=dmask,
                compare_op=mybir.AluOpType.not_equal,
                fill=DIAG_NEG,
                base=0,
                pattern=[[-1, P]],
                channel_multiplier=1,
            )

            wq_sb = singles.tile([D, DK], F32)
            nc.sync.dma_start(wq_sb, wq_d[:, :])
            wk_sb = singles.tile([D, DK], F32)
            nc.sync.dma_start(wk_sb, wk_d[:, :])

            for b in range(BPC):
                # --- load query[b] as [p, n, d], s = n*128 + p ---
                q_sb = qload.tile([P, NQ, D], F32)
                nc.sync.dma_start(
                    q_sb, q_d[b].rearrange("(n p) d -> p n d", p=P)
                )

                # --- transpose to queryT [d, (n p)] = [128, 1024] ---
                qT = qtp.tile([D, NQ, P], F32)
                for n in range(NQ):
                    tp_ps = ps_tp.tile([P, P], F32)
                    nc.tensor.transpose(tp_ps, q_sb[:, n], ident)
                    nc.vector.tensor_copy(qT[:, n], tp_ps)

                # --- projections Q/K = W.T @ queryT into one fp32 psum tile:
                # Q on partitions 0:64, K on 64:128. Then split into bf16
                # hi+lo for fast near-fp32 scores:
                #   scores = Qh.T Kh + Ql.T Kh + Qh.T Kl   (Ql.T Kl dropped)
                # All split ops use only fast DVE paths (fp32 TT, casts).
                pp = ps_proj.tile([P, S], F32, tag="proj")
                nc.tensor.matmul(pp[0:DK, 0:512], wq_sb, qT[:, 0:4])
                nc.tensor.matmul(pp[0:DK, 512:1024], wq_sb, qT[:, 4:8])
                nc.tensor.matmul(pp[DK:P, 0:512], wk_sb, qT[:, 0:4])
                nc.tensor.matmul(pp[DK:P, 512:1024], wk_sb, qT[:, 4:8])

                hb = projsb.tile([P, S], BF16, tag="hb")    # [Qh; Kh]
                nc.vector.tensor_copy(hb, pp)
                h32 = projsb.tile([P, S], F32, tag="h32")
                nc.vector.tensor_copy(h32, hb)
                lb = projsb.tile([P, S], BF16, tag="lb")    # [Ql; Kl]
                nc.vector.tensor_tensor(lb, pp, h32, mybir.AluOpType.subtract)

                qstack = projsb.tile([P, S], BF16, tag="qstack")  # [Qh; Ql]
                nc.vector.tensor_copy(qstack[0:DK], hb[0:DK])
                nc.vector.tensor_copy(qstack[DK:P], lb[0:DK])
                khh = projsb.tile([P, S], BF16, tag="khh")        # [Kh; Kh]
                nc.vector.tensor_copy(khh[0:DK], hb[DK:P])
                nc.vector.tensor_copy(khh[DK:P], hb[DK:P])
                kl = projsb.tile([DK, S], BF16, tag="kl")         # Kl
                nc.vector.tensor_copy(kl, lb[DK:P])

                # --- scores + tanh per 128-row chunk ---
                t_sb = tbuf.tile([P, NQ, S], F32, tag="t")
                for qc in range(NQ):
                    sc_ps = ps_sc.tile([P, S], F32, tag="sc")
                    lhsT = qstack[:, qc * P:(qc + 1) * P]
                    lhsT_h = qstack[0:DK, qc * P:(qc + 1) * P]
                    for h in range(2):
                        cols = slice(h * 512, (h + 1) * 512)
                        nc.tensor.matmul(
                            sc_ps[:, cols], lhsT, khh[:, cols],
                            start=True, stop=False,
                        )
                        nc.tensor.matmul(
                            sc_ps[:, cols], lhsT_h, kl[:, cols],
                            start=False, stop=True,
                        )
                    nc.scalar.activation(
                        out=t_sb[:, qc],
                        in_=sc_ps,
                        func=mybir.ActivationFunctionType.Tanh,
                    )
                    # clamp this chunk's diagonal block to -1e4
                    blk = t_sb[:, qc, qc * P:(qc + 1) * P]
                    nc.vector.tensor_tensor(blk, blk, dmask, mybir.AluOpType.min)

                # --- exp(10*t) in place, with per-partition row sums ---
                rs = small.tile([P, 1], F32, tag="rs")
                nc.scalar.activation(
                    out=t_sb,
                    in_=t_sb,
                    func=mybir.ActivationFunctionType.Exp,
                    scale=TANH_CLIP,
                    accum_out=rs,
                )

                # --- Z = sum over partitions; rz = 1/Z broadcast [128,1] ---
                zall = small.tile([P, 1], F32, tag="zall")
                nc.gpsimd.partition_all_reduce(
                    zall, rs, channels=P, reduce_op=bass_isa.ReduceOp.add
                )
                rz = small.tile([P, 1], F32, tag="rz")
                nc.vector.reciprocal(rz, zall)

                # --- normalize in place and store ---
                nc.vector.tensor_scalar_mul(t_sb, t_sb, rz)
                nc.sync.dma_start(
                    out_d[b].rearrange("(n p) s -> p n s", p=P), t_sb
                )

    nc.compile()
    return nc


_CACHED_NC = None


def kernel(**inputs: np.ndarray) -> np.ndarray:
    global _CACHED_NC
    query = np.ascontiguousarray(np.asarray(inputs["query"], dtype=np.float32))
    wq = np.ascontiguousarray(np.asarray(inputs["W_query"], dtype=np.float32))
    wk = np.ascontiguousarray(np.asarray(inputs["W_key"], dtype=np.float32))
    assert query.shape == (B, S, D), query.shape

    if _CACHED_NC is None:
        _CACHED_NC = build_bass()
    nc = _CACHED_NC

    in_maps = [
        {
            "query": query[c * BPC:(c + 1) * BPC],
            "W_query": wq,
            "W_key": wk,
        }
        for c in range(N_CORES)
    ]
    res = run_bass_kernel_spmd(nc, in_maps, core_ids=list(range(N_CORES)))
    out = np.concatenate(
        [r["out"].reshape(BPC, S * S) for r in res.results], axis=0
    )
    return out
